# revision 2
# baseline (speedup 1.0000x reference)
"""KANLinear forward on 8 Trainium2 cores.

Math: spline bases via truncated-power identity
  bases_k(x) = (1/6) sum_{m=0..4} (-1)^m C(4,m) relu(y - (k+m))^3,  y = (x+2.2)/0.4
The banded (1,-4,6,-4,1)/6 combination is folded into the spline weights on
the host, so the device computes only 12 shifted relu-cubes r_j = relu(y-j)^3
plus silu(x), then one fused matmul over contraction (j,i) + (base branch).

Data-parallel: x sharded along batch over 8 cores, weights replicated.
"""
import numpy as np

import concourse.bass as bass
import concourse.tile as tile
import concourse.mybir as mybir
from concourse import bacc
from concourse.bass_utils import run_bass_kernel_spmd
from concourse.masks import make_identity

F32 = mybir.dt.float32
F16 = mybir.dt.float16
AF = mybir.ActivationFunctionType
ALU = mybir.AluOpType

B, IN, OUT, NCOEF = 32768, 256, 256, 8
NCORES = 8
B_CORE = B // NCORES          # 4096
ST = 512                      # supertile batch rows
NST = B_CORE // ST            # 8
NJ = 12                       # truncated-power slices
GRID0, H = -2.2, 0.4          # grid[0], spacing
SCALE = 1.0 / H               # 2.5
BIAS = -GRID0 / H             # 5.5

_CACHE = {}


def _build_nc():
    nc = bacc.Bacc(None, target_bir_lowering=False)
    x_in = nc.dram_tensor("x", [B_CORE, IN], F32, kind="ExternalInput")
    wpt_in = nc.dram_tensor("wpt", [NJ, IN, OUT], F16, kind="ExternalInput")
    bwt_in = nc.dram_tensor("bwt", [IN, OUT], F16, kind="ExternalInput")
    out_d = nc.dram_tensor("out", [B_CORE, OUT], F32, kind="ExternalOutput")

    with tile.TileContext(nc) as tc:
        with tc.tile_pool(name="wpool", bufs=1) as wpool, \
             tc.tile_pool(name="xpool", bufs=3) as xpool, \
             tc.tile_pool(name="ypool", bufs=2) as ypool, \
             tc.tile_pool(name="vpool", bufs=4) as vpool, \
             tc.tile_pool(name="spool", bufs=4) as spool, \
             tc.tile_pool(name="rpool", bufs=2) as rpool, \
             tc.tile_pool(name="opool", bufs=3) as opool, \
             tc.tile_pool(name="xtps", bufs=2, space="PSUM") as xtps, \
             tc.tile_pool(name="ops", bufs=3, space="PSUM") as opsp:

            # --- one-time: weights, identity, bias consts ---
            ident = wpool.tile([128, 128], F32, tag="ident", name="ident")
            make_identity(nc, ident)

            w_sb = [[wpool.tile([128, OUT], F16, tag=f"w{j}_{ih}", name=f"w{j}_{ih}")
                     for ih in range(2)] for j in range(NJ)]
            for j in range(NJ):
                for ih in range(2):
                    nc.sync.dma_start(out=w_sb[j][ih],
                                      in_=wpt_in[j, ih * 128:(ih + 1) * 128, :])
            bw_sb = [wpool.tile([128, OUT], F16, tag=f"bw{ih}", name=f"bw{ih}") for ih in range(2)]
            for ih in range(2):
                nc.sync.dma_start(out=bw_sb[ih],
                                  in_=bwt_in[ih * 128:(ih + 1) * 128, :])
            # per-j bias tiles for ACT Square: value (BIAS - j)
            bias_t = [wpool.tile([128, 1], F32, tag=f"b{j}", name=f"b{j}") for j in range(NJ)]
            for j in range(NJ):
                nc.gpsimd.memset(bias_t[j], BIAS - float(j))

            # engine split for s (v^2) and r (s*v)
            S_ON_ACT = {(j, ih) for j in (0, 2, 4, 6, 8, 10) for ih in range(2)}
            R_ON_GPS = {(j, ih) for j in (1, 3, 5, 7) for ih in range(2)}

            for st in range(NST):
                b0 = st * ST
                xt = [xtps.tile([128, ST], F32, tag=f"xt{ih}", name=f"xt{ih}") for ih in range(2)]
                for q in range(4):
                    x_sb = xpool.tile([128, IN], F32, tag="x", name="x_sb")
                    nc.sync.dma_start(out=x_sb,
                                      in_=x_in[b0 + q * 128: b0 + (q + 1) * 128, :])
                    for ih in range(2):
                        nc.tensor.transpose(
                            xt[ih][:, q * 128:(q + 1) * 128],
                            x_sb[:, ih * 128:(ih + 1) * 128], ident)

                silu = []
                ys = []
                for ih in range(2):
                    s_t = ypool.tile([128, ST], F16, tag=f"silu{ih}", name=f"silu{ih}")
                    nc.scalar.activation(s_t, xt[ih], AF.Silu)
                    silu.append(s_t)
                    y_t = ypool.tile([128, ST], F16, tag=f"y{ih}", name=f"y{ih}")
                    nc.scalar.activation(y_t, xt[ih], AF.Copy,
                                         bias=BIAS, scale=SCALE)
                    ys.append(y_t)

                r_t = [[None] * 2 for _ in range(NJ)]
                for j in range(NJ):
                    for ih in range(2):
                        v = vpool.tile([128, ST], F16, tag="v", name="v")
                        nc.vector.tensor_scalar(v, ys[ih], float(j), 0.0,
                                                ALU.subtract, ALU.max)
                        s = spool.tile([128, ST], F16, tag="s", name="s")
                        if (j, ih) in S_ON_ACT:
                            nc.scalar.activation(s, xt[ih], AF.Square,
                                                 bias=bias_t[j], scale=SCALE)
                        else:
                            nc.vector.tensor_mul(s, v, v)
                        r = rpool.tile([128, ST], F16, tag=f"r{j}_{ih}", name=f"r{j}_{ih}")
                        if (j, ih) in R_ON_GPS:
                            nc.gpsimd.tensor_mul(r, s, v)
                        else:
                            nc.vector.tensor_mul(r, s, v)
                        r_t[j][ih] = r

                for q in range(4):
                    qs = slice(q * 128, (q + 1) * 128)
                    ops = opsp.tile([128, OUT], F32, tag="ops", name="ops")
                    n_mm = 2 + 2 * NJ
                    i_mm = 0
                    for ih in range(2):
                        nc.tensor.matmul(ops, silu[ih][:, qs], bw_sb[ih],
                                         start=(i_mm == 0),
                                         stop=(i_mm == n_mm - 1))
                        i_mm += 1
                    for j in range(NJ):
                        for ih in range(2):
                            nc.tensor.matmul(ops, r_t[j][ih][:, qs], w_sb[j][ih],
                                             start=(i_mm == 0),
                                             stop=(i_mm == n_mm - 1))
                            i_mm += 1
                    osb = opool.tile([128, OUT], F32, tag="osb", name="osb")
                    nc.scalar.copy(osb, ops)
                    nc.sync.dma_start(
                        out=out_d[b0 + q * 128: b0 + (q + 1) * 128, :], in_=osb)

    nc.finalize()
    return nc


def _prep_weights(base_weight, spline_weight, spline_scaler):
    c = np.array([1.0, -4.0, 6.0, -4.0, 1.0], dtype=np.float64) / 6.0
    w_scaled = spline_weight.astype(np.float64) * \
        spline_scaler.astype(np.float64)[..., None]          # [O, I, 8]
    wpt = np.zeros((NJ, IN, OUT), dtype=np.float64)          # [j, i, o]
    for j in range(NJ):
        for m in range(5):
            k = j - m
            if 0 <= k < NCOEF:
                wpt[j] += c[m] * w_scaled[:, :, k].T
    return wpt.astype(np.float16), base_weight.T.astype(np.float16)


def kernel(x, base_weight, spline_weight, spline_scaler, grid):
    if "nc" not in _CACHE:
        _CACHE["nc"] = _build_nc()
    nc = _CACHE["nc"]
    wpt, bwt = _prep_weights(base_weight, spline_weight, spline_scaler)
    in_maps = [{"x": np.ascontiguousarray(x[c * B_CORE:(c + 1) * B_CORE]),
                "wpt": wpt, "bwt": bwt} for c in range(NCORES)]
    res = run_bass_kernel_spmd(nc, in_maps, core_ids=list(range(NCORES)))
    return np.concatenate([r["out"] for r in res.results], axis=0)
